# revision 18
# baseline (speedup 1.0000x reference)
"""Trainium2 Bass kernel: 2-layer GraphSAGE (degree-normalized mean aggregation,
self-loops) + elementwise-product link-prediction MLP.

Distribution (8 NeuronCores):
  - Nodes sharded contiguously across cores (12544-row padded shards).
  - Edges sharded by RECEIVER core, sorted by (receiver block, sender chunk);
    per-core segment sums computed locally with an indicator-matmul trick
    (one-hot(edge->slot) matrices built on DVE, reduced on the PE), so no
    cross-core reduction is needed -- just AllGathers of each layer's node
    table, split into 4 chunk-pieces so downstream gathers can start as soon
    as their chunk lands.
  - Pairs sharded by the a-endpoint's home core; a-rows are expanded from the
    LOCAL h shard with one-hot matmuls on the PE (no DMA descriptors), only
    the b-side goes through dma_gather.  Pair slots are grouped into
    statically-sized cells per (b-chunk, a-block) so the instruction stream
    is identical across cores (SPMD) while contents differ.
Gathers use the SWDGE dma_gather custom instruction (int16 indices relative to
one of 4 table chunks of <=32K rows).  SWDGE descriptor generation on the Q7
is the critical resource (~7.5ns/row), which is why the a-side avoids it.
"""

import os
import sys

import numpy as np

_TRN_REPO = "/opt/trn_rl_repo"
if _TRN_REPO not in sys.path:
    sys.path.insert(0, _TRN_REPO)

import ml_dtypes

BF16 = ml_dtypes.bfloat16

# ---------------------------------------------------------------- problem cfg
R = 8  # cores
D = 128  # feature dim
N = int(os.environ.get("GNN_N", 100000))

NIDX_TILES = 32  # max 128-idx tiles per dma_gather call (4096 rows = 1MB bf16)
GSUP = 6  # blocks per supergroup (PSUM: one bank per block + 1 tr + 1 h)

NLOC = N // R
NB = -(-NLOC // 128)  # node blocks per core
SHARD = NB * 128
TAB = R * SHARD
NCHUNK = 4
PIECE = SHARD // NCHUNK
CHUNK = R * PIECE  # = TAB // NCHUNK
PCELL = 16  # pair-cell slot granularity
assert N % R == 0 and CHUNK <= 32767 and SHARD % NCHUNK == 0

_TRACE = False
_LAST_EXEC_NS = None
_LAST_RESULTS = None


def _cdiv(a, b):
    return -(-a // b)


def _trow(n):
    """Node id -> row in the piece-major AllGather'd table."""
    c = n // NLOC
    i = n % NLOC
    return (i // PIECE) * CHUNK + c * PIECE + (i % PIECE)


# ---------------------------------------------------------------- host prep
def _wrap16(idx_stream):
    """int16 idx stream (len = m*128) -> [128, m*8] wrapped-16 layout."""
    m8 = len(idx_stream) // 16
    a = idx_stream.reshape(m8, 16).T  # [16, m*8]
    return np.tile(a, (8, 1)).astype(np.int16)


def _preprocess(senders, receivers, pairs):
    s = np.concatenate([senders.astype(np.int64), np.arange(N, dtype=np.int64)])
    r = np.concatenate([receivers.astype(np.int64), np.arange(N, dtype=np.int64)])

    deg = np.bincount(s, minlength=N).astype(np.float64)
    cnt = np.bincount(r, minlength=N).astype(np.float64)
    ssend_n = (1.0 / np.sqrt(np.maximum(deg, 1.0))).astype(np.float32)
    srecv_n = (np.maximum(cnt, 1.0) ** -1.5).astype(np.float32)

    def pad_shard(v):
        out = np.zeros((R, SHARD), np.float32)
        for c in range(R):
            out[c, :NLOC] = v[c * NLOC : (c + 1) * NLOC]
        return out

    ssend_sh = pad_shard(ssend_n)
    srecv_sh = pad_shard(srecv_n)

    # self-loop contributions are applied densely on-device (identity matmul),
    # so only real edges go through the gather stream
    se = senders.astype(np.int64)
    re = receivers.astype(np.int64)
    srow = _trow(se)
    rcore = re // NLOC
    rloc = re % NLOC
    eblk = rloc // 128
    eslot = rloc % 128
    ech = srow // CHUNK
    erel = (srow % CHUNK).astype(np.int64)

    # supergroups of blocks
    groups = [list(range(g, min(g + GSUP, NB))) for g in range(0, NB, GSUP)]
    sgid = np.zeros(NB, np.int64)
    for gi, g in enumerate(groups):
        for b in g:
            sgid[b] = gi

    nbuck = NB * NCHUNK
    key = eblk * NCHUNK + ech
    counts = np.zeros((R, nbuck), np.int64)
    for c in range(R):
        counts[c] = np.bincount(key[rcore == c], minlength=nbuck)
    tiles_bc = _cdiv(counts.max(axis=0), 128).reshape(NB, NCHUNK)

    # static call plan -------------------------------------------------------
    # stream order: for sg: for chunk: for b in sg: bucket(b, chunk)
    layer_calls = []  # dicts: sg, chunk, ntiles, tile_blocks, tcol
    blk_total = {b: int(tiles_bc[b].sum()) for b in range(NB)}
    tcursor = 0
    for gi, g in enumerate(groups):
        for c in range(NCHUNK):
            tile_blocks = []
            for b in g:
                tile_blocks += [b] * int(tiles_bc[b, c])
            pos = 0
            while pos < len(tile_blocks):
                m = min(NIDX_TILES, len(tile_blocks) - pos)
                layer_calls.append(
                    dict(
                        sg=gi,
                        chunk=c,
                        ntiles=m,
                        tile_blocks=tile_blocks[pos : pos + m],
                        tcol=tcursor,
                    )
                )
                tcursor += m
                pos += m
    GT = tcursor  # total tiles per layer stream

    # per-core edge streams --------------------------------------------------
    gidx = np.zeros((R, 128, GT * 8), np.int16)
    gseg = np.full((R, 128, GT), -1.0, dtype=BF16)
    order_key = sgid[eblk] * (NCHUNK * (NB + 1)) + ech * (NB + 1) + eblk
    for c in range(R):
        m = rcore == c
        ords = np.lexsort((np.zeros(m.sum()), order_key[m]))
        ce_rel = erel[m][ords]
        ce_slot = eslot[m][ords]
        ce_key = key[m][ords]
        cnts = np.bincount(ce_key, minlength=nbuck)
        border = []
        for gi, g in enumerate(groups):
            for ch in range(NCHUNK):
                for b in g:
                    border.append((b, ch))
        sort_off = 0
        idx_stream = np.zeros(GT * 128, np.int64)
        seg_stream = np.full(GT * 128, -1.0, np.float32)
        out_off = 0
        for (b, ch) in border:
            k = b * NCHUNK + ch
            n = int(cnts[k])
            nt = int(tiles_bc[b, ch])
            idx_stream[out_off : out_off + n] = ce_rel[sort_off : sort_off + n]
            seg_stream[out_off : out_off + n] = ce_slot[sort_off : sort_off + n]
            sort_off += n
            out_off += nt * 128
        assert out_off == GT * 128 and sort_off == m.sum()
        gidx[c] = _wrap16(idx_stream.astype(np.int16))
        gseg[c] = seg_stream.reshape(GT, 128).T.astype(BF16)

    # pairs ------------------------------------------------------------------
    # Shard by a's home core; expand a from the local h shard on the PE.
    # Slots grouped into cells per (b-chunk, a-block); cell sizes are static
    # (max over cores, rounded to PCELL) so the call plan is SPMD-uniform.
    P = pairs.shape[0]
    pa = pairs[:, 0].astype(np.int64)
    pb = pairs[:, 1].astype(np.int64)
    pacore = pa // NLOC
    pb_tr = _trow(pb)
    pbch = pb_tr // CHUNK
    pbrel = pb_tr % CHUNK
    al = pa % NLOC
    ablk = al // 128
    aslot = al % 128

    cellkey = pbch * NB + ablk  # (bchunk, ablk)
    ncell = NCHUNK * NB
    ccnt = np.zeros((R, ncell), np.int64)
    for c in range(R):
        ccnt[c] = np.bincount(cellkey[pacore == c], minlength=ncell)
    cell_sz = _cdiv(ccnt.max(axis=0), PCELL) * PCELL  # static per-cell slots

    # stream layout: bchunk-major, ablk-minor; each bchunk padded to 128
    cell_off = np.zeros(ncell, np.int64)
    chunk_tile0 = []  # first tile of each bchunk
    chunk_ntiles = []
    off = 0
    for ch in range(NCHUNK):
        chunk_tile0.append(off // 128)
        for b in range(NB):
            k = ch * NB + b
            cell_off[k] = off
            off += int(cell_sz[k])
        off = _cdiv(off, 128) * 128
        chunk_ntiles.append(off // 128 - chunk_tile0[-1])
    PTp = off // 128  # total pair tiles

    # incidences: per tile, the (ablk, oh-index) list of overlapping cells
    tile_incs = [[] for _ in range(PTp)]
    ninc = 0
    for ch in range(NCHUNK):
        for b in range(NB):
            k = ch * NB + b
            if cell_sz[k] == 0:
                continue
            t0 = cell_off[k] // 128
            t1 = (cell_off[k] + cell_sz[k] - 1) // 128
            for t in range(t0, t1 + 1):
                tile_incs[t].append((b, ninc))
                ninc += 1
    NINC = ninc

    # pair calls: per bchunk, batches of <= NIDX_TILES tiles
    pair_calls = []
    for ch in range(NCHUNK):
        pos = 0
        while pos < chunk_ntiles[ch]:
            m = min(NIDX_TILES, chunk_ntiles[ch] - pos)
            t0 = chunk_tile0[ch] + pos
            incs = []
            for t in range(t0, t0 + m):
                incs.append(tile_incs[t])
            all_i = [i for x in incs for (_, i) in x]
            inc0 = min(all_i) if all_i else 0
            nincs = len(all_i)
            assert not all_i or max(all_i) - inc0 + 1 == nincs
            pair_calls.append(
                dict(chunk=ch, ntiles=m, tcol=t0, incs=incs, inc0=inc0, nincs=nincs)
            )
            pos += m

    # per-core pair streams
    pbidx = np.zeros((R, 128, PTp * 8), np.int16)
    paoh = np.zeros((R, 128, NINC * 128), dtype=BF16)
    posmap = np.full((R, PTp * 128), -1, np.int64)
    for c in range(R):
        ids = np.nonzero(pacore == c)[0]
        b_stream = np.zeros(PTp * 128, np.int64)
        a_slot_stream = np.full(PTp * 128, -1, np.int64)
        for k in range(ncell):
            sel = ids[cellkey[ids] == k]
            n = len(sel)
            o = cell_off[k]
            b_stream[o : o + n] = pbrel[sel]
            a_slot_stream[o : o + n] = aslot[sel]
            posmap[c, o : o + n] = sel
        pbidx[c] = _wrap16(b_stream.astype(np.int16))
        # build one-hot matrices per incidence
        inc_i = 0
        for ch in range(NCHUNK):
            for b in range(NB):
                k = ch * NB + b
                if cell_sz[k] == 0:
                    continue
                t0 = cell_off[k] // 128
                t1 = (cell_off[k] + cell_sz[k] - 1) // 128
                for t in range(t0, t1 + 1):
                    lo = max(cell_off[k], t * 128)
                    hi = min(cell_off[k] + cell_sz[k], (t + 1) * 128)
                    M = np.zeros((128, 128), np.float32)
                    for pos in range(lo, hi):
                        sl = a_slot_stream[pos]
                        if sl >= 0:
                            M[sl, pos - t * 128] = 1.0
                    paoh[c][:, inc_i * 128 : (inc_i + 1) * 128] = M.astype(BF16)
                    inc_i += 1
        assert inc_i == NINC

    meta = dict(
        groups=groups,
        tiles_bc=tiles_bc,
        blk_total=blk_total,
        layer_calls=layer_calls,
        GT=GT,
        pair_calls=pair_calls,
        PTp=PTp,
        NINC=NINC,
    )
    data = dict(
        ssend_sh=ssend_sh,
        srecv_sh=srecv_sh,
        gidx=gidx,
        gseg=gseg,
        pbidx=pbidx,
        paoh=paoh,
        posmap=posmap,
    )
    return meta, data


# ---------------------------------------------------------------- bass build
def _build(meta, bb_val):
    from concourse import bass, mybir, bacc
    import concourse.tile as tile
    from concourse.masks import make_identity

    f32 = mybir.dt.float32
    bf16 = mybir.dt.bfloat16
    i16 = mybir.dt.int16
    GT = meta["GT"]
    PTp = meta["PTp"]
    NINC = meta["NINC"]
    groups = meta["groups"]
    blk_total = meta["blk_total"]

    nc = bacc.Bacc(
        "TRN2",
        target_bir_lowering=False,
        debug=False,
        num_devices=R,
        num_swdge_queues=4,
    )

    emb_s = nc.dram_tensor("emb_s", [SHARD, D], bf16, kind="ExternalInput")
    ssend_t = nc.dram_tensor("ssend", [128, NB], f32, kind="ExternalInput")
    srecv_t = nc.dram_tensor("srecv", [128, NB], f32, kind="ExternalInput")
    gidx_t = nc.dram_tensor("gidx", [128, GT * 8], i16, kind="ExternalInput")
    gseg_t = nc.dram_tensor("gseg", [128, GT], bf16, kind="ExternalInput")
    pbidx_t = nc.dram_tensor("pbidx", [128, PTp * 8], i16, kind="ExternalInput")
    paoh_t = nc.dram_tensor("paoh", [128, NINC * 128], bf16, kind="ExternalInput")
    w1t_t = nc.dram_tensor("w1t", [D, D], bf16, kind="ExternalInput")
    w1b_t = nc.dram_tensor("w1b", [D, D], bf16, kind="ExternalInput")
    w2t_t = nc.dram_tensor("w2t", [D, D], bf16, kind="ExternalInput")
    w2b_t = nc.dram_tensor("w2b", [D, D], bf16, kind="ExternalInput")
    wa_t = nc.dram_tensor("wa", [D, D], bf16, kind="ExternalInput")
    wb_t = nc.dram_tensor("wb", [D, 1], bf16, kind="ExternalInput")
    b1_t = nc.dram_tensor("b1", [1, D], bf16, kind="ExternalInput")
    b2_t = nc.dram_tensor("b2", [1, D], bf16, kind="ExternalInput")
    ba_t = nc.dram_tensor("ba", [D, 1], f32, kind="ExternalInput")
    iota_in = nc.dram_tensor("iota", [128, 128], bf16, kind="ExternalInput")
    out_t = nc.dram_tensor("scores", [PTp * 128], f32, kind="ExternalOutput")

    rg = [list(range(R))]
    eq = mybir.AluOpType.is_equal
    amax = mybir.AluOpType.max
    amul = mybir.AluOpType.mult
    aadd = mybir.AluOpType.add

    def g3(ap, m):
        return ap.rearrange("p (t d) -> p t d", d=128)

    with tile.TileContext(nc) as tc:
        with (
            tc.tile_pool(name="const", bufs=1) as cp,
            tc.tile_pool(name="dram", bufs=1, space="DRAM") as dp,
        ):
            w1t = cp.tile([D, D], bf16)
            nc.sync.dma_start(w1t[:, :], w1t_t[:, :])
            w1b = cp.tile([D, D], bf16)
            nc.sync.dma_start(w1b[:, :], w1b_t[:, :])
            w2t = cp.tile([D, D], bf16)
            nc.sync.dma_start(w2t[:, :], w2t_t[:, :])
            w2b = cp.tile([D, D], bf16)
            nc.sync.dma_start(w2b[:, :], w2b_t[:, :])
            wa = cp.tile([D, D], bf16)
            nc.sync.dma_start(wa[:, :], wa_t[:, :])
            wb = cp.tile([D, 1], bf16)
            nc.sync.dma_start(wb[:, :], wb_t[:, :])
            b1 = cp.tile([1, D], bf16)
            nc.sync.dma_start(b1[:, :], b1_t[:, :])
            b2 = cp.tile([1, D], bf16)
            nc.sync.dma_start(b2[:, :], b2_t[:, :])
            ba = cp.tile([D, 1], f32)
            nc.sync.dma_start(ba[:, :], ba_t[:, :])
            iota = cp.tile([128, 128], bf16)
            nc.sync.dma_start(iota[:, :], iota_in[:, :])
            ones1 = cp.tile([1, 128], bf16)
            nc.vector.memset(ones1[:, :], 1.0)
            ident = cp.tile([128, 128], f32)
            make_identity(nc, ident[:, :])
            identb = cp.tile([128, 128], bf16)
            nc.vector.tensor_copy(identb[:, :], ident[:, :])

            tab1 = [
                dp.tile([CHUNK, D], bf16, addr_space="Shared", name=f"tab1_{p}")
                for p in range(NCHUNK)
            ]
            tab2 = [
                dp.tile([CHUNK, D], bf16, addr_space="Shared", name=f"tab2_{p}")
                for p in range(NCHUNK)
            ]
            tab3 = [
                dp.tile([CHUNK, D], bf16, addr_space="Shared", name=f"tab3_{p}")
                for p in range(NCHUNK)
            ]
            ag1in = [dp.tile([PIECE, D], bf16, name=f"ag1i_{p}") for p in range(NCHUNK)]
            ag2in = [dp.tile([PIECE, D], bf16, name=f"ag2i_{p}") for p in range(NCHUNK)]
            ag3in = [dp.tile([PIECE, D], bf16, name=f"ag3i_{p}") for p in range(NCHUNK)]
            x2s = dp.tile([SHARD, D], bf16)

            def store_block(pieces, b, sb_tile):
                """Store sbuf tile [128, D] to shard rows [b*128,(b+1)*128) of
                piece-split dram tiles (handles piece straddles)."""
                r0 = b * 128
                while r0 < (b + 1) * 128:
                    p = r0 // PIECE
                    r1 = min((b + 1) * 128, (p + 1) * PIECE)
                    nc.sync.dma_start(
                        pieces[p][r0 - p * PIECE : r1 - p * PIECE, :],
                        sb_tile[r0 - b * 128 : r1 - b * 128, :],
                    )
                    r0 = r1

            def load_block(pieces, b, sb_tile):
                r0 = b * 128
                while r0 < (b + 1) * 128:
                    p = r0 // PIECE
                    r1 = min((b + 1) * 128, (p + 1) * PIECE)
                    nc.sync.dma_start(
                        sb_tile[r0 - b * 128 : r1 - b * 128, :],
                        pieces[p][r0 - p * PIECE : r1 - p * PIECE, :],
                    )
                    r0 = r1

            def emit_ag_piece(agin, tab, p):
                nc.gpsimd.collective_compute(
                    "AllGather",
                    mybir.AluOpType.bypass,
                    replica_groups=rg,
                    ins=[agin[p][:, :].opt()],
                    outs=[tab[p][:, :].opt()],
                )

            # ---- P1: xnorm1 = emb * ssend (one bulk load + one DVE op),
            #      store per piece and fire that piece's AllGather right away
            with tc.tile_pool(name="p1", bufs=1) as p1:
                xall = p1.tile([128, NB * 128], bf16)
                nc.sync.dma_start(
                    xall.rearrange("p (b d) -> p b d", d=128),
                    emb_s.rearrange("(b p) d -> p b d", p=128),
                )
                stf = p1.tile([128, NB], f32)
                nc.sync.dma_start(stf[:, :], ssend_t[:, :])
                stb = p1.tile([128, NB], bf16)
                nc.vector.tensor_copy(stb[:, :], stf[:, :])
                xnall = p1.tile([128, NB * 128], bf16)
                nc.vector.tensor_tensor(
                    out=xnall.rearrange("p (b d) -> p b d", d=128),
                    in0=xall.rearrange("p (b d) -> p b d", d=128),
                    in1=stb[:, :].to_broadcast([128, NB, 128]),
                    op=amul,
                )
                for pp in range(NCHUNK):
                    r0 = pp * PIECE
                    pend = (pp + 1) * PIECE
                    while r0 < pend:
                        if r0 % 128 == 0 and pend - r0 >= 128:
                            nbf = (pend - r0) // 128
                            b0 = r0 // 128
                            nc.sync.dma_start(
                                ag1in[pp][
                                    r0 - pp * PIECE : r0 - pp * PIECE + nbf * 128, :
                                ].rearrange("(b p) d -> p b d", p=128),
                                xnall[:, b0 * 128 : (b0 + nbf) * 128].rearrange(
                                    "p (b d) -> p b d", d=128
                                ),
                            )
                            r0 += nbf * 128
                        else:
                            b = r0 // 128
                            r1 = min(pend, (b + 1) * 128)
                            nc.sync.dma_start(
                                ag1in[pp][r0 - pp * PIECE : r1 - pp * PIECE, :],
                                xnall[r0 - b * 128 : r1 - b * 128, b * 128 : (b + 1) * 128],
                            )
                            r0 = r1
                    emit_ag_piece(ag1in, tab1, pp)

            # ---- layers
            gq = [0]

            def next_queue():
                q = (gq[0] // 2) % 4
                gq[0] += 1
                return q

            def emit_layer(tab, xnsrc, x_src_whole, x_src_pieces, wtop, wbot, bias,
                           relu, x2_out, agin_out, tab_next):
                ag_fired = [False] * NCHUNK
                with (
                    tc.tile_pool(name="gat", bufs=3) as gp,
                    tc.tile_pool(name="ind", bufs=2) as ip,
                    tc.tile_pool(name="gmeta", bufs=4) as mp,
                    tc.tile_pool(name="epi", bufs=3) as ep,
                    tc.tile_pool(name="xup", bufs=GSUP) as xp,
                    tc.tile_pool(name="agg", bufs=GSUP, space="PSUM") as aggp,
                    tc.tile_pool(name="trp", bufs=1, space="PSUM") as trp,
                    tc.tile_pool(name="hp", bufs=1, space="PSUM") as hp,
                ):
                    call_i = 0
                    for gi, g in enumerate(groups):
                        aggt = [
                            aggp.tile([128, 128], f32, tag="aggt", name=f"aggt{k}")
                            for k in range(len(g))
                        ]
                        done = {b: 0 for b in g}
                        for j, b in enumerate(g):
                            xnb = ep.tile([128, D], bf16, tag="xnb")
                            load_block(xnsrc, b, xnb)
                            nc.tensor.matmul(
                                aggt[j][:, :],
                                lhsT=identb[:, :],
                                rhs=xnb[:, :],
                                start=True,
                                stop=(blk_total[b] == 0),
                            )
                        while call_i < len(meta["layer_calls"]) and meta["layer_calls"][call_i]["sg"] == gi:
                            call = meta["layer_calls"][call_i]
                            call_i += 1
                            m = call["ntiles"]
                            c = call["chunk"]
                            t0 = call["tcol"]
                            idx = mp.tile([128, m * 8], i16, tag="idx")
                            nc.sync.dma_start(idx[:, :], gidx_t[:, t0 * 8 : (t0 + m) * 8])
                            seg = mp.tile([128, m], bf16, tag="seg")
                            nc.sync.dma_start(seg[:, :], gseg_t[:, t0 : t0 + m])
                            gat = gp.tile([128, m * 128], bf16, tag="gat")
                            nc.gpsimd.dma_gather(
                                g3(gat[:, :], m),
                                tab[c][:, :],
                                idx[:, :],
                                m * 128,
                                m * 128,
                                D,
                                single_packet=False,
                                queue_num=next_queue(),
                            )
                            ind = ip.tile([128, m * 128], bf16, tag="ind")
                            nc.vector.tensor_tensor(
                                out=g3(ind[:, :], m),
                                in0=seg[:, :].to_broadcast([128, m, 128]),
                                in1=iota[:, :]
                                .rearrange("p (t d) -> p t d", t=1)
                                .to_broadcast([128, m, 128]),
                                op=eq,
                            )
                            for tpos, b in enumerate(call["tile_blocks"]):
                                done[b] += 1
                                last = done[b] == blk_total[b]
                                j = g.index(b)
                                nc.tensor.matmul(
                                    aggt[j][:, :],
                                    lhsT=ind[:, tpos * 128 : (tpos + 1) * 128],
                                    rhs=gat[:, tpos * 128 : (tpos + 1) * 128],
                                    start=False,
                                    stop=last,
                                )
                        # epilogue: first drain all PSUM aggregators (quick
                        # scales) so the next supergroup's gathers/seeds can
                        # proceed, then the heavy per-block tail
                        xupds = []
                        for j, b in enumerate(g):
                            srv = mp.tile([128, 1], f32, tag="srv")
                            nc.sync.dma_start(srv[:, :], srecv_t[:, b : b + 1])
                            xupd = xp.tile([128, D], bf16, tag="xupd")
                            nc.vector.tensor_scalar_mul(xupd[:, :], aggt[j][:, :], srv[:, :])
                            xupds.append(xupd)
                        for j, b in enumerate(g):
                            xupd = xupds[j]
                            ps1 = trp.tile([128, 128], bf16, tag="tr")
                            nc.tensor.transpose(ps1[:, :], xupd[:, :], identb[:, :])
                            xupdT = ep.tile([128, D], bf16, tag="xupdT")
                            nc.vector.tensor_copy(xupdT[:, :], ps1[:, :])
                            xe = ep.tile([128, D], bf16, tag="xe2")
                            if x_src_whole is not None:
                                nc.sync.dma_start(
                                    xe[:, :], x_src_whole[b * 128 : (b + 1) * 128, :]
                                )
                            else:
                                load_block(x_src_pieces, b, xe)
                            ps2 = trp.tile([128, 128], bf16, tag="tr")
                            nc.tensor.transpose(ps2[:, :], xe[:, :], identb[:, :])
                            xT = ep.tile([128, D], bf16, tag="xT")
                            nc.vector.tensor_copy(xT[:, :], ps2[:, :])
                            hps = hp.tile([128, 128], f32, tag="h")
                            nc.tensor.matmul(hps[:, :], lhsT=xT[:, :], rhs=wtop[:, :], start=True, stop=False)
                            nc.tensor.matmul(hps[:, :], lhsT=xupdT[:, :], rhs=wbot[:, :], start=False, stop=False)
                            nc.tensor.matmul(hps[:, :], lhsT=ones1[:, :], rhs=bias[:, :], start=False, stop=True)
                            if relu:
                                hx = ep.tile([128, D], bf16, tag="hx")
                                nc.vector.tensor_scalar_max(hx[:, :], hps[:, :], 0.0)
                                nc.sync.dma_start(x2_out[b * 128 : (b + 1) * 128, :], hx[:, :])
                                ssd = mp.tile([128, 1], f32, tag="ssd")
                                nc.sync.dma_start(ssd[:, :], ssend_t[:, b : b + 1])
                                xn2 = ep.tile([128, D], bf16, tag="xn2")
                                nc.vector.tensor_scalar(
                                    xn2[:, :],
                                    hps[:, :],
                                    0.0,
                                    ssd[:, :],
                                    op0=amax,
                                    op1=amul,
                                )
                                store_block(agin_out, b, xn2)
                            else:
                                hx = ep.tile([128, D], bf16, tag="hxb")
                                nc.vector.tensor_copy(hx[:, :], hps[:, :])
                                store_block(agin_out, b, hx)
                        # fire next-table AllGather pieces whose producer
                        # blocks are all epilogued, overlapping the transfer
                        # behind the remaining gathers of this layer
                        for p in range(NCHUNK):
                            if not ag_fired[p] and ((p + 1) * PIECE - 1) // 128 <= g[-1]:
                                emit_ag_piece(agin_out, tab_next, p)
                                ag_fired[p] = True
                assert all(ag_fired)

            emit_layer(tab1, ag1in, emb_s, None, w1t, w1b, b1, True, x2s, ag2in, tab2)
            emit_layer(tab2, ag2in, x2s, None, w2t, w2b, b2, False, None, ag3in, tab3)

            # ---- pairs: a expanded from local h shard via PE one-hots,
            #      b gathered from the AllGather'd table
            with (
                tc.tile_pool(name="hres", bufs=1) as hrp,
                tc.tile_pool(name="pgat", bufs=3) as gp,
                tc.tile_pool(name="pz", bufs=2) as zp,
                tc.tile_pool(name="pmeta", bufs=4) as mp,
                tc.tile_pool(name="poh", bufs=2) as ohp,
                tc.tile_pool(name="pepi", bufs=4) as ep,
                tc.tile_pool(name="pap", bufs=2, space="PSUM") as pap,
                tc.tile_pool(name="pzt", bufs=2, space="PSUM") as ztp,
                tc.tile_pool(name="pza", bufs=2, space="PSUM") as zap,
                tc.tile_pool(name="psc", bufs=2, space="PSUM") as scp,
            ):
                hres = hrp.tile([128, NB * 128], bf16)
                for b in range(NB):
                    r0 = b * 128
                    while r0 < (b + 1) * 128:
                        p = r0 // PIECE
                        r1 = min((b + 1) * 128, (p + 1) * PIECE)
                        nc.sync.dma_start(
                            hres[r0 - b * 128 : r1 - b * 128, b * 128 : (b + 1) * 128],
                            ag3in[p][r0 - p * PIECE : r1 - p * PIECE, :],
                        )
                        r0 = r1

                for call in meta["pair_calls"]:
                    m = call["ntiles"]
                    t0 = call["tcol"]
                    inc0 = call["inc0"]
                    nincs = call["nincs"]
                    ib = mp.tile([128, m * 8], i16, tag="pib")
                    nc.sync.dma_start(ib[:, :], pbidx_t[:, t0 * 8 : (t0 + m) * 8])
                    gb = gp.tile([128, m * 128], bf16, tag="gb")
                    nc.gpsimd.dma_gather(
                        g3(gb[:, :], m),
                        tab3[call["chunk"]][:, :],
                        ib[:, :],
                        m * 128,
                        m * 128,
                        D,
                        single_packet=False,
                        queue_num=next_queue(),
                    )
                    if nincs > 0:
                        ohc = ohp.tile([128, nincs * 128], bf16, tag="ohc")
                        nc.sync.dma_start(
                            ohc[:, :], paoh_t[:, inc0 * 128 : (inc0 + nincs) * 128]
                        )
                    for bt in range(0, m, 4):
                        nb = min(4, m - bt)
                        z = zp.tile([128, 512], bf16, tag="z")
                        for i in range(nb):
                            incs = call["incs"][bt + i]
                            pa_ps = pap.tile([128, 128], f32, tag="pa")
                            for k, (ab, inc_i) in enumerate(incs):
                                nc.tensor.matmul(
                                    pa_ps[:, :],
                                    lhsT=ohc[:, (inc_i - inc0) * 128 : (inc_i - inc0 + 1) * 128],
                                    rhs=hres[:, ab * 128 : (ab + 1) * 128],
                                    start=(k == 0),
                                    stop=(k == len(incs) - 1),
                                )
                            ha = ep.tile([128, 128], bf16, tag="ha")
                            if incs:
                                nc.vector.tensor_copy(ha[:, :], pa_ps[:, :])
                            else:
                                nc.vector.memset(ha[:, :], 0.0)
                            nc.vector.tensor_mul(
                                z[:, i * 128 : (i + 1) * 128],
                                ha[:, :],
                                gb[:, (bt + i) * 128 : (bt + i + 1) * 128],
                            )
                        zt_ps = ztp.tile([128, 512], bf16, tag="zt")
                        for i in range(nb):
                            nc.tensor.matmul(
                                zt_ps[:, i * 128 : (i + 1) * 128],
                                lhsT=z[:, i * 128 : (i + 1) * 128],
                                rhs=identb[:, :],
                                is_transpose=True,
                                start=(i == 0),
                                stop=(i == nb - 1),
                            )
                        zt = ep.tile([128, 512], bf16, tag="zts")
                        nc.vector.tensor_copy(zt[:, : nb * 128], zt_ps[:, : nb * 128])
                        za_ps = zap.tile([128, 512], f32, tag="za")
                        for i in range(nb):
                            nc.tensor.matmul(
                                za_ps[:, i * 128 : (i + 1) * 128],
                                lhsT=wa[:, :],
                                rhs=zt[:, i * 128 : (i + 1) * 128],
                                start=(i == 0),
                                stop=(i == nb - 1),
                            )
                        za = ep.tile([128, 512], bf16, tag="zas")
                        nc.vector.tensor_scalar(
                            za[:, : nb * 128],
                            za_ps[:, : nb * 128],
                            ba[:, :],
                            0.0,
                            op0=aadd,
                            op1=amax,
                        )
                        sc_ps = scp.tile([1, 512], f32, tag="sc")
                        for i in range(nb):
                            nc.tensor.matmul(
                                sc_ps[:, i * 128 : (i + 1) * 128],
                                lhsT=wb[:, :],
                                rhs=za[:, i * 128 : (i + 1) * 128],
                                start=(i == 0),
                                stop=(i == nb - 1),
                            )
                        sc = ep.tile([1, 512], f32, tag="scs")
                        nc.vector.tensor_scalar_add(
                            sc[:, : nb * 128], sc_ps[:, : nb * 128], float(bb_val)
                        )
                        o0 = (t0 + bt) * 128
                        nc.sync.dma_start(
                            out_t[o0 : o0 + nb * 128].rearrange("(x n) -> x n", x=1),
                            sc[:, : nb * 128],
                        )
    nc.compile()
    return nc


# ---------------------------------------------------------------- entry point
def kernel(
    node_ids,
    senders,
    receivers,
    pairs,
    emb,
    W1,
    b1,
    W2,
    b2,
    Wa,
    ba,
    Wb,
    bb,
):
    global _LAST_EXEC_NS, _LAST_RESULTS
    from concourse import bass_utils

    node_ids = np.asarray(node_ids)
    senders = np.asarray(senders).astype(np.int64)
    receivers = np.asarray(receivers).astype(np.int64)
    pairs_np = np.asarray(pairs).astype(np.int64)
    emb = np.asarray(emb, dtype=np.float32)
    W1 = np.asarray(W1, dtype=np.float32)
    b1 = np.asarray(b1, dtype=np.float32)
    W2 = np.asarray(W2, dtype=np.float32)
    b2 = np.asarray(b2, dtype=np.float32)
    Wa = np.asarray(Wa, dtype=np.float32)
    ba = np.asarray(ba, dtype=np.float32)
    Wb = np.asarray(Wb, dtype=np.float32)
    bb = np.asarray(bb, dtype=np.float32)

    # x = emb[node_ids]
    x0 = emb[np.asarray(node_ids).astype(np.int64)]

    meta, data = _preprocess(senders, receivers, pairs_np)
    nc = _build(meta, float(bb.reshape(-1)[0]))

    iota = np.tile(np.arange(128, dtype=np.float32), (128, 1)).astype(BF16)
    in_maps = []
    for c in range(R):
        emb_sh = np.zeros((SHARD, D), BF16)
        emb_sh[:NLOC] = x0[c * NLOC : (c + 1) * NLOC].astype(BF16)
        in_maps.append(
            dict(
                emb_s=emb_sh,
                ssend=np.ascontiguousarray(data["ssend_sh"][c].reshape(NB, 128).T),
                srecv=np.ascontiguousarray(data["srecv_sh"][c].reshape(NB, 128).T),
                gidx=data["gidx"][c],
                gseg=data["gseg"][c],
                pbidx=data["pbidx"][c],
                paoh=data["paoh"][c],
                w1t=np.ascontiguousarray(W1[:D]).astype(BF16),
                w1b=np.ascontiguousarray(W1[D:]).astype(BF16),
                w2t=np.ascontiguousarray(W2[:D]).astype(BF16),
                w2b=np.ascontiguousarray(W2[D:]).astype(BF16),
                wa=Wa.astype(BF16),
                wb=Wb.astype(BF16),
                b1=b1.reshape(1, D).astype(BF16),
                b2=b2.reshape(1, D).astype(BF16),
                ba=ba.reshape(D, 1),
                iota=iota,
            )
        )

    res = bass_utils.run_bass_kernel_spmd(
        nc,
        in_maps,
        core_ids=list(range(R)),
        trace=bool(_TRACE),
        trace_cores=[0] if _TRACE == "light" else None,
    )
    _LAST_EXEC_NS = res.exec_time_ns
    _LAST_RESULTS = res

    P = pairs_np.shape[0]
    scores = np.zeros(P, np.float32)
    for c in range(R):
        v = np.asarray(res.results[c]["scores"])
        pm = data["posmap"][c]
        mvalid = pm >= 0
        scores[pm[mvalid]] = v[mvalid]
    return scores


# revision 24
# speedup vs baseline: 1.1077x; 1.1077x over previous
"""Trainium2 Bass kernel: 2-layer GraphSAGE (degree-normalized mean aggregation,
self-loops) + elementwise-product link-prediction MLP.

Distribution (8 NeuronCores):
  - Nodes sharded contiguously across cores (12544-row padded shards).
  - Edges sharded by RECEIVER core, sorted by (receiver block, sender chunk);
    per-core segment sums computed locally with an indicator-matmul trick
    (one-hot(edge->slot) matrices built on DVE, reduced on the PE), so no
    cross-core reduction is needed -- just AllGathers of each layer's node
    table, split into 4 chunk-pieces so downstream gathers can start as soon
    as their chunk lands.
  - Pairs sharded by the a-endpoint's home core; a-rows are expanded from the
    LOCAL h shard with one-hot matmuls on the PE (no DMA descriptors), only
    the b-side goes through dma_gather.  Pair slots are grouped into
    statically-sized cells per (b-chunk, a-block) so the instruction stream
    is identical across cores (SPMD) while contents differ.
Gathers use the SWDGE dma_gather custom instruction (int16 indices relative to
one of 4 table chunks of <=32K rows).  SWDGE descriptor generation on the Q7
is the critical resource (~7.5ns/row), which is why the a-side avoids it.
"""

import os
import sys

import numpy as np

_TRN_REPO = "/opt/trn_rl_repo"
if _TRN_REPO not in sys.path:
    sys.path.insert(0, _TRN_REPO)

import ml_dtypes

BF16 = ml_dtypes.bfloat16

# ---------------------------------------------------------------- problem cfg
R = 8  # cores
D = 128  # feature dim
N = int(os.environ.get("GNN_N", 100000))

NIDX_TILES = 32  # max 128-idx tiles per dma_gather call (4096 rows = 1MB bf16)
GSUP = 6  # blocks per supergroup (PSUM: one bank per block + 1 tr + 1 h)

NLOC = N // R
NB = -(-NLOC // 128)  # node blocks per core
SHARD = NB * 128
TAB = R * SHARD
NCHUNK = 4
PIECE = SHARD // NCHUNK
CHUNK = R * PIECE  # = TAB // NCHUNK
PCELL = 16  # pair-cell slot granularity
assert N % R == 0 and CHUNK <= 32767 and SHARD % NCHUNK == 0

_TRACE = False
_LAST_EXEC_NS = None
_LAST_RESULTS = None


def _cdiv(a, b):
    return -(-a // b)


def _trow(n):
    """Node id -> row in the piece-major AllGather'd table."""
    c = n // NLOC
    i = n % NLOC
    return (i // PIECE) * CHUNK + c * PIECE + (i % PIECE)


# ---------------------------------------------------------------- host prep
def _wrap16(idx_stream):
    """int16 idx stream (len = m*128) -> [128, m*8] wrapped-16 layout."""
    m8 = len(idx_stream) // 16
    a = idx_stream.reshape(m8, 16).T  # [16, m*8]
    return np.tile(a, (8, 1)).astype(np.int16)


def _preprocess(senders, receivers, pairs):
    s = np.concatenate([senders.astype(np.int64), np.arange(N, dtype=np.int64)])
    r = np.concatenate([receivers.astype(np.int64), np.arange(N, dtype=np.int64)])

    deg = np.bincount(s, minlength=N).astype(np.float64)
    cnt = np.bincount(r, minlength=N).astype(np.float64)
    ssend_n = (1.0 / np.sqrt(np.maximum(deg, 1.0))).astype(np.float32)
    srecv_n = (np.maximum(cnt, 1.0) ** -1.5).astype(np.float32)

    def pad_shard(v):
        out = np.zeros((R, SHARD), np.float32)
        for c in range(R):
            out[c, :NLOC] = v[c * NLOC : (c + 1) * NLOC]
        return out

    ssend_sh = pad_shard(ssend_n)
    srecv_sh = pad_shard(srecv_n)

    # self-loop contributions are applied densely on-device (identity matmul),
    # so only real edges go through the gather stream
    se = senders.astype(np.int64)
    re = receivers.astype(np.int64)
    srow = _trow(se)
    rcore = re // NLOC
    rloc = re % NLOC
    eblk = rloc // 128
    eslot = rloc % 128
    ech = srow // CHUNK
    erel = (srow % CHUNK).astype(np.int64)

    # supergroups of blocks
    groups = [list(range(g, min(g + GSUP, NB))) for g in range(0, NB, GSUP)]
    sgid = np.zeros(NB, np.int64)
    for gi, g in enumerate(groups):
        for b in g:
            sgid[b] = gi

    nbuck = NB * NCHUNK
    key = eblk * NCHUNK + ech
    counts = np.zeros((R, nbuck), np.int64)
    for c in range(R):
        counts[c] = np.bincount(key[rcore == c], minlength=nbuck)
    tiles_bc = _cdiv(counts.max(axis=0), 128).reshape(NB, NCHUNK)

    # static call plan -------------------------------------------------------
    # stream order: for sg: for chunk: for b in sg: bucket(b, chunk)
    layer_calls = []  # dicts: sg, chunk, ntiles, tile_blocks, tcol
    blk_total = {b: int(tiles_bc[b].sum()) for b in range(NB)}
    tcursor = 0
    for gi, g in enumerate(groups):
        for c in range(NCHUNK):
            tile_blocks = []
            for b in g:
                tile_blocks += [b] * int(tiles_bc[b, c])
            pos = 0
            while pos < len(tile_blocks):
                m = min(NIDX_TILES, len(tile_blocks) - pos)
                layer_calls.append(
                    dict(
                        sg=gi,
                        chunk=c,
                        ntiles=m,
                        tile_blocks=tile_blocks[pos : pos + m],
                        tcol=tcursor,
                    )
                )
                tcursor += m
                pos += m
    GT = tcursor  # total tiles per layer stream

    # per-core edge streams --------------------------------------------------
    gidx = np.zeros((R, 128, GT * 8), np.int16)
    gseg = np.full((R, 128, GT), -1.0, dtype=BF16)
    order_key = sgid[eblk] * (NCHUNK * (NB + 1)) + ech * (NB + 1) + eblk
    for c in range(R):
        m = rcore == c
        ords = np.lexsort((np.zeros(m.sum()), order_key[m]))
        ce_rel = erel[m][ords]
        ce_slot = eslot[m][ords]
        ce_key = key[m][ords]
        cnts = np.bincount(ce_key, minlength=nbuck)
        border = []
        for gi, g in enumerate(groups):
            for ch in range(NCHUNK):
                for b in g:
                    border.append((b, ch))
        sort_off = 0
        idx_stream = np.zeros(GT * 128, np.int64)
        seg_stream = np.full(GT * 128, -1.0, np.float32)
        out_off = 0
        for (b, ch) in border:
            k = b * NCHUNK + ch
            n = int(cnts[k])
            nt = int(tiles_bc[b, ch])
            idx_stream[out_off : out_off + n] = ce_rel[sort_off : sort_off + n]
            seg_stream[out_off : out_off + n] = ce_slot[sort_off : sort_off + n]
            sort_off += n
            out_off += nt * 128
        assert out_off == GT * 128 and sort_off == m.sum()
        gidx[c] = _wrap16(idx_stream.astype(np.int16))
        gseg[c] = seg_stream.reshape(GT, 128).T.astype(BF16)

    # pairs ------------------------------------------------------------------
    # Shard by a's home core; expand a from the local h shard on the PE.
    # Slots grouped into cells per (b-chunk, a-block); cell sizes are static
    # (max over cores, rounded to PCELL) so the call plan is SPMD-uniform.
    P = pairs.shape[0]
    pa = pairs[:, 0].astype(np.int64)
    pb = pairs[:, 1].astype(np.int64)
    pacore = pa // NLOC
    pb_tr = _trow(pb)
    pbch = pb_tr // CHUNK
    pbrel = pb_tr % CHUNK
    al = pa % NLOC
    ablk = al // 128
    aslot = al % 128

    cellkey = pbch * NB + ablk  # (bchunk, ablk)
    ncell = NCHUNK * NB
    ccnt = np.zeros((R, ncell), np.int64)
    for c in range(R):
        ccnt[c] = np.bincount(cellkey[pacore == c], minlength=ncell)
    cell_sz = _cdiv(ccnt.max(axis=0), PCELL) * PCELL  # static per-cell slots

    # stream layout: bchunk-major, ablk-minor; each bchunk padded to 128
    cell_off = np.zeros(ncell, np.int64)
    chunk_tile0 = []  # first tile of each bchunk
    chunk_ntiles = []
    off = 0
    for ch in range(NCHUNK):
        chunk_tile0.append(off // 128)
        for b in range(NB):
            k = ch * NB + b
            cell_off[k] = off
            off += int(cell_sz[k])
        off = _cdiv(off, 128) * 128
        chunk_ntiles.append(off // 128 - chunk_tile0[-1])
    PTp = off // 128  # total pair tiles

    # incidences: per tile, the (ablk, oh-index) list of overlapping cells
    tile_incs = [[] for _ in range(PTp)]
    ninc = 0
    for ch in range(NCHUNK):
        for b in range(NB):
            k = ch * NB + b
            if cell_sz[k] == 0:
                continue
            t0 = cell_off[k] // 128
            t1 = (cell_off[k] + cell_sz[k] - 1) // 128
            for t in range(t0, t1 + 1):
                tile_incs[t].append((b, ninc))
                ninc += 1
    NINC = ninc

    # pair calls: per bchunk, batches of <= NIDX_TILES tiles
    pair_calls = []
    for ch in range(NCHUNK):
        pos = 0
        while pos < chunk_ntiles[ch]:
            m = min(NIDX_TILES, chunk_ntiles[ch] - pos)
            t0 = chunk_tile0[ch] + pos
            incs = []
            for t in range(t0, t0 + m):
                incs.append(tile_incs[t])
            all_i = [i for x in incs for (_, i) in x]
            inc0 = min(all_i) if all_i else 0
            nincs = len(all_i)
            assert not all_i or max(all_i) - inc0 + 1 == nincs
            pair_calls.append(
                dict(chunk=ch, ntiles=m, tcol=t0, incs=incs, inc0=inc0, nincs=nincs)
            )
            pos += m

    # per-core pair streams
    pbidx = np.zeros((R, 128, PTp * 8), np.int16)
    paoh = np.zeros((R, 128, NINC * 128), dtype=BF16)
    posmap = np.full((R, PTp * 128), -1, np.int64)
    for c in range(R):
        ids = np.nonzero(pacore == c)[0]
        b_stream = np.zeros(PTp * 128, np.int64)
        a_slot_stream = np.full(PTp * 128, -1, np.int64)
        for k in range(ncell):
            sel = ids[cellkey[ids] == k]
            n = len(sel)
            o = cell_off[k]
            b_stream[o : o + n] = pbrel[sel]
            a_slot_stream[o : o + n] = aslot[sel]
            posmap[c, o : o + n] = sel
        pbidx[c] = _wrap16(b_stream.astype(np.int16))
        # build one-hot matrices per incidence
        inc_i = 0
        for ch in range(NCHUNK):
            for b in range(NB):
                k = ch * NB + b
                if cell_sz[k] == 0:
                    continue
                t0 = cell_off[k] // 128
                t1 = (cell_off[k] + cell_sz[k] - 1) // 128
                for t in range(t0, t1 + 1):
                    lo = max(cell_off[k], t * 128)
                    hi = min(cell_off[k] + cell_sz[k], (t + 1) * 128)
                    M = np.zeros((128, 128), np.float32)
                    for pos in range(lo, hi):
                        sl = a_slot_stream[pos]
                        if sl >= 0:
                            M[sl, pos - t * 128] = 1.0
                    paoh[c][:, inc_i * 128 : (inc_i + 1) * 128] = M.astype(BF16)
                    inc_i += 1
        assert inc_i == NINC

    meta = dict(
        groups=groups,
        tiles_bc=tiles_bc,
        blk_total=blk_total,
        layer_calls=layer_calls,
        GT=GT,
        pair_calls=pair_calls,
        PTp=PTp,
        NINC=NINC,
    )
    data = dict(
        ssend_sh=ssend_sh,
        srecv_sh=srecv_sh,
        gidx=gidx,
        gseg=gseg,
        pbidx=pbidx,
        paoh=paoh,
        posmap=posmap,
    )
    return meta, data


# ---------------------------------------------------------------- bass build
def _build(meta, bb_val):
    from concourse import bass, mybir, bacc
    import concourse.tile as tile
    from concourse.masks import make_identity

    f32 = mybir.dt.float32
    bf16 = mybir.dt.bfloat16
    i16 = mybir.dt.int16
    GT = meta["GT"]
    PTp = meta["PTp"]
    NINC = meta["NINC"]
    groups = meta["groups"]
    blk_total = meta["blk_total"]

    nc = bacc.Bacc(
        "TRN2",
        target_bir_lowering=False,
        debug=False,
        num_devices=R,
        num_swdge_queues=4,
    )

    emb_s = nc.dram_tensor("emb_s", [SHARD, D], bf16, kind="ExternalInput")
    ssend_t = nc.dram_tensor("ssend", [128, NB], f32, kind="ExternalInput")
    srecv_t = nc.dram_tensor("srecv", [128, NB], f32, kind="ExternalInput")
    gidx_t = nc.dram_tensor("gidx", [128, GT * 8], i16, kind="ExternalInput")
    gseg_t = nc.dram_tensor("gseg", [128, GT], bf16, kind="ExternalInput")
    pbidx_t = nc.dram_tensor("pbidx", [128, PTp * 8], i16, kind="ExternalInput")
    paoh_t = nc.dram_tensor("paoh", [128, NINC * 128], bf16, kind="ExternalInput")
    w1t_t = nc.dram_tensor("w1t", [D, D], bf16, kind="ExternalInput")
    w1b_t = nc.dram_tensor("w1b", [D, D], bf16, kind="ExternalInput")
    w2t_t = nc.dram_tensor("w2t", [D, D], bf16, kind="ExternalInput")
    w2b_t = nc.dram_tensor("w2b", [D, D], bf16, kind="ExternalInput")
    wa_t = nc.dram_tensor("wa", [D, D], bf16, kind="ExternalInput")
    wb_t = nc.dram_tensor("wb", [D, 1], bf16, kind="ExternalInput")
    b1_t = nc.dram_tensor("b1", [1, D], bf16, kind="ExternalInput")
    b2_t = nc.dram_tensor("b2", [1, D], bf16, kind="ExternalInput")
    ba_t = nc.dram_tensor("ba", [D, 1], f32, kind="ExternalInput")
    iota_in = nc.dram_tensor("iota", [128, 128], bf16, kind="ExternalInput")
    out_t = nc.dram_tensor("scores", [PTp * 128], f32, kind="ExternalOutput")

    rg = [list(range(R))]
    eq = mybir.AluOpType.is_equal
    amax = mybir.AluOpType.max
    amul = mybir.AluOpType.mult
    aadd = mybir.AluOpType.add

    def g3(ap, m):
        return ap.rearrange("p (t d) -> p t d", d=128)

    with tile.TileContext(nc) as tc:
        with (
            tc.tile_pool(name="const", bufs=1) as cp,
            tc.tile_pool(name="dram", bufs=1, space="DRAM") as dp,
        ):
            w1t = cp.tile([D, D], bf16)
            nc.sync.dma_start(w1t[:, :], w1t_t[:, :])
            w1b = cp.tile([D, D], bf16)
            nc.sync.dma_start(w1b[:, :], w1b_t[:, :])
            w2t = cp.tile([D, D], bf16)
            nc.sync.dma_start(w2t[:, :], w2t_t[:, :])
            w2b = cp.tile([D, D], bf16)
            nc.sync.dma_start(w2b[:, :], w2b_t[:, :])
            wa = cp.tile([D, D], bf16)
            nc.sync.dma_start(wa[:, :], wa_t[:, :])
            wb = cp.tile([D, 1], bf16)
            nc.sync.dma_start(wb[:, :], wb_t[:, :])
            b1 = cp.tile([1, D], bf16)
            nc.sync.dma_start(b1[:, :], b1_t[:, :])
            b2 = cp.tile([1, D], bf16)
            nc.sync.dma_start(b2[:, :], b2_t[:, :])
            ba = cp.tile([D, 1], f32)
            nc.sync.dma_start(ba[:, :], ba_t[:, :])
            iota = cp.tile([128, 128], bf16)
            nc.sync.dma_start(iota[:, :], iota_in[:, :])
            ones1 = cp.tile([1, 128], bf16)
            nc.vector.memset(ones1[:, :], 1.0)
            ident = cp.tile([128, 128], f32)
            make_identity(nc, ident[:, :])
            identb = cp.tile([128, 128], bf16)
            nc.vector.tensor_copy(identb[:, :], ident[:, :])

            tab1 = [
                dp.tile([CHUNK, D], bf16, addr_space="Shared", name=f"tab1_{p}")
                for p in range(NCHUNK)
            ]
            tab2 = [
                dp.tile([CHUNK, D], bf16, addr_space="Shared", name=f"tab2_{p}")
                for p in range(NCHUNK)
            ]
            tab3 = [
                dp.tile([CHUNK, D], bf16, addr_space="Shared", name=f"tab3_{p}")
                for p in range(NCHUNK)
            ]
            ag1in = [dp.tile([PIECE, D], bf16, name=f"ag1i_{p}") for p in range(NCHUNK)]
            ag2in = [dp.tile([PIECE, D], bf16, name=f"ag2i_{p}") for p in range(NCHUNK)]
            ag3in = [dp.tile([PIECE, D], bf16, name=f"ag3i_{p}") for p in range(NCHUNK)]
            x2s = dp.tile([SHARD, D], bf16)

            def store_block(pieces, b, sb_tile):
                """Store sbuf tile [128, D] to shard rows [b*128,(b+1)*128) of
                piece-split dram tiles (handles piece straddles)."""
                r0 = b * 128
                while r0 < (b + 1) * 128:
                    p = r0 // PIECE
                    r1 = min((b + 1) * 128, (p + 1) * PIECE)
                    nc.sync.dma_start(
                        pieces[p][r0 - p * PIECE : r1 - p * PIECE, :],
                        sb_tile[r0 - b * 128 : r1 - b * 128, :],
                    )
                    r0 = r1

            def load_block(pieces, b, sb_tile):
                r0 = b * 128
                while r0 < (b + 1) * 128:
                    p = r0 // PIECE
                    r1 = min((b + 1) * 128, (p + 1) * PIECE)
                    nc.sync.dma_start(
                        sb_tile[r0 - b * 128 : r1 - b * 128, :],
                        pieces[p][r0 - p * PIECE : r1 - p * PIECE, :],
                    )
                    r0 = r1

            def emit_ag_piece(agin, tab, p):
                nc.gpsimd.collective_compute(
                    "AllGather",
                    mybir.AluOpType.bypass,
                    replica_groups=rg,
                    ins=[agin[p][:, :].opt()],
                    outs=[tab[p][:, :].opt()],
                )

            # ---- P1: xnorm1 = emb * ssend (one bulk load + one DVE op),
            #      store per piece and fire that piece's AllGather right away
            with tc.tile_pool(name="p1", bufs=1) as p1:
                xall = p1.tile([128, NB * 128], bf16)
                nc.sync.dma_start(
                    xall.rearrange("p (b d) -> p b d", d=128),
                    emb_s.rearrange("(b p) d -> p b d", p=128),
                )
                stf = p1.tile([128, NB], f32)
                nc.sync.dma_start(stf[:, :], ssend_t[:, :])
                stb = p1.tile([128, NB], bf16)
                nc.vector.tensor_copy(stb[:, :], stf[:, :])
                xnall = p1.tile([128, NB * 128], bf16)
                nc.vector.tensor_tensor(
                    out=xnall.rearrange("p (b d) -> p b d", d=128),
                    in0=xall.rearrange("p (b d) -> p b d", d=128),
                    in1=stb[:, :].to_broadcast([128, NB, 128]),
                    op=amul,
                )
                for pp in range(NCHUNK):
                    r0 = pp * PIECE
                    pend = (pp + 1) * PIECE
                    while r0 < pend:
                        if r0 % 128 == 0 and pend - r0 >= 128:
                            nbf = (pend - r0) // 128
                            b0 = r0 // 128
                            nc.sync.dma_start(
                                ag1in[pp][
                                    r0 - pp * PIECE : r0 - pp * PIECE + nbf * 128, :
                                ].rearrange("(b p) d -> p b d", p=128),
                                xnall[:, b0 * 128 : (b0 + nbf) * 128].rearrange(
                                    "p (b d) -> p b d", d=128
                                ),
                            )
                            r0 += nbf * 128
                        else:
                            b = r0 // 128
                            r1 = min(pend, (b + 1) * 128)
                            nc.sync.dma_start(
                                ag1in[pp][r0 - pp * PIECE : r1 - pp * PIECE, :],
                                xnall[r0 - b * 128 : r1 - b * 128, b * 128 : (b + 1) * 128],
                            )
                            r0 = r1
                    emit_ag_piece(ag1in, tab1, pp)

            # ---- layers
            gq = [0]

            def next_queue():
                q = (gq[0] // 2) % 4
                gq[0] += 1
                return q

            def emit_layer(tab, xnsrc, x_src_whole, x_src_pieces, wtop, wbot, bias,
                           relu, x2_out, agin_out, tab_next):
                ag_fired = [False] * NCHUNK
                with (
                    tc.tile_pool(name="gat", bufs=5) as gp,
                    tc.tile_pool(name="ind", bufs=3) as ip,
                    tc.tile_pool(name="gmeta", bufs=6) as mp,
                    tc.tile_pool(name="epi", bufs=3) as ep,
                    tc.tile_pool(name="agg", bufs=GSUP, space="PSUM") as aggp,
                    tc.tile_pool(name="trp", bufs=1, space="PSUM") as trp,
                    tc.tile_pool(name="hp", bufs=1, space="PSUM") as hp,
                ):
                    call_i = 0
                    for gi, g in enumerate(groups):
                        aggt = [
                            aggp.tile([128, 128], f32, tag="aggt", name=f"aggt{k}")
                            for k in range(len(g))
                        ]
                        done = {b: 0 for b in g}
                        for j, b in enumerate(g):
                            xnb = ep.tile([128, D], bf16, tag="xnb")
                            load_block(xnsrc, b, xnb)
                            nc.tensor.matmul(
                                aggt[j][:, :],
                                lhsT=identb[:, :],
                                rhs=xnb[:, :],
                                start=True,
                                stop=(blk_total[b] == 0),
                            )
                        while call_i < len(meta["layer_calls"]) and meta["layer_calls"][call_i]["sg"] == gi:
                            call = meta["layer_calls"][call_i]
                            call_i += 1
                            m = call["ntiles"]
                            c = call["chunk"]
                            t0 = call["tcol"]
                            idx = mp.tile([128, m * 8], i16, tag="idx")
                            nc.sync.dma_start(idx[:, :], gidx_t[:, t0 * 8 : (t0 + m) * 8])
                            seg = mp.tile([128, m], bf16, tag="seg")
                            nc.sync.dma_start(seg[:, :], gseg_t[:, t0 : t0 + m])
                            gat = gp.tile([128, m * 128], bf16, tag="gat")
                            nc.gpsimd.dma_gather(
                                g3(gat[:, :], m),
                                tab[c][:, :],
                                idx[:, :],
                                m * 128,
                                m * 128,
                                D,
                                single_packet=False,
                                queue_num=next_queue(),
                            )
                            ind = ip.tile([128, m * 128], bf16, tag="ind")
                            nc.vector.tensor_tensor(
                                out=g3(ind[:, :], m),
                                in0=seg[:, :].to_broadcast([128, m, 128]),
                                in1=iota[:, :]
                                .rearrange("p (t d) -> p t d", t=1)
                                .to_broadcast([128, m, 128]),
                                op=eq,
                            )
                            for tpos, b in enumerate(call["tile_blocks"]):
                                done[b] += 1
                                last = done[b] == blk_total[b]
                                j = g.index(b)
                                nc.tensor.matmul(
                                    aggt[j][:, :],
                                    lhsT=ind[:, tpos * 128 : (tpos + 1) * 128],
                                    rhs=gat[:, tpos * 128 : (tpos + 1) * 128],
                                    start=False,
                                    stop=last,
                                )
                        # epilogue per block
                        for j, b in enumerate(g):
                            srv = mp.tile([128, 1], f32, tag="srv")
                            nc.sync.dma_start(srv[:, :], srecv_t[:, b : b + 1])
                            xupd = ep.tile([128, D], bf16, tag="xupd")
                            nc.vector.tensor_scalar_mul(xupd[:, :], aggt[j][:, :], srv[:, :])
                            ps1 = trp.tile([128, 128], bf16, tag="tr")
                            nc.tensor.transpose(ps1[:, :], xupd[:, :], identb[:, :])
                            xupdT = ep.tile([128, D], bf16, tag="xupdT")
                            nc.vector.tensor_copy(xupdT[:, :], ps1[:, :])
                            xe = ep.tile([128, D], bf16, tag="xe2")
                            if x_src_whole is not None:
                                nc.sync.dma_start(
                                    xe[:, :], x_src_whole[b * 128 : (b + 1) * 128, :]
                                )
                            else:
                                load_block(x_src_pieces, b, xe)
                            ps2 = trp.tile([128, 128], bf16, tag="tr")
                            nc.tensor.transpose(ps2[:, :], xe[:, :], identb[:, :])
                            xT = ep.tile([128, D], bf16, tag="xT")
                            nc.vector.tensor_copy(xT[:, :], ps2[:, :])
                            hps = hp.tile([128, 128], f32, tag="h")
                            nc.tensor.matmul(hps[:, :], lhsT=xT[:, :], rhs=wtop[:, :], start=True, stop=False)
                            nc.tensor.matmul(hps[:, :], lhsT=xupdT[:, :], rhs=wbot[:, :], start=False, stop=False)
                            nc.tensor.matmul(hps[:, :], lhsT=ones1[:, :], rhs=bias[:, :], start=False, stop=True)
                            if relu:
                                hx = ep.tile([128, D], bf16, tag="hx")
                                nc.vector.tensor_scalar_max(hx[:, :], hps[:, :], 0.0)
                                nc.sync.dma_start(x2_out[b * 128 : (b + 1) * 128, :], hx[:, :])
                                ssd = mp.tile([128, 1], f32, tag="ssd")
                                nc.sync.dma_start(ssd[:, :], ssend_t[:, b : b + 1])
                                xn2 = ep.tile([128, D], bf16, tag="xn2")
                                nc.vector.tensor_scalar(
                                    xn2[:, :],
                                    hps[:, :],
                                    0.0,
                                    ssd[:, :],
                                    op0=amax,
                                    op1=amul,
                                )
                                store_block(agin_out, b, xn2)
                            else:
                                hx = ep.tile([128, D], bf16, tag="hxb")
                                nc.vector.tensor_copy(hx[:, :], hps[:, :])
                                store_block(agin_out, b, hx)
                        # fire next-table AllGather pieces whose producer
                        # blocks are all epilogued, overlapping the transfer
                        # behind the remaining gathers of this layer
                        for p in range(NCHUNK):
                            if not ag_fired[p] and ((p + 1) * PIECE - 1) // 128 <= g[-1]:
                                emit_ag_piece(agin_out, tab_next, p)
                                ag_fired[p] = True
                assert all(ag_fired)

            emit_layer(tab1, ag1in, emb_s, None, w1t, w1b, b1, True, x2s, ag2in, tab2)
            emit_layer(tab2, ag2in, x2s, None, w2t, w2b, b2, False, None, ag3in, tab3)

            # ---- pairs: a expanded from local h shard via PE one-hots,
            #      b gathered from the AllGather'd table
            with (
                tc.tile_pool(name="hres", bufs=1) as hrp,
                tc.tile_pool(name="pgat", bufs=3) as gp,
                tc.tile_pool(name="pz", bufs=2) as zp,
                tc.tile_pool(name="pmeta", bufs=4) as mp,
                tc.tile_pool(name="poh", bufs=2) as ohp,
                tc.tile_pool(name="pepi", bufs=4) as ep,
                tc.tile_pool(name="pap", bufs=2, space="PSUM") as pap,
                tc.tile_pool(name="pzt", bufs=2, space="PSUM") as ztp,
                tc.tile_pool(name="pza", bufs=2, space="PSUM") as zap,
                tc.tile_pool(name="psc", bufs=2, space="PSUM") as scp,
            ):
                hres = hrp.tile([128, NB * 128], bf16)
                for b in range(NB):
                    r0 = b * 128
                    while r0 < (b + 1) * 128:
                        p = r0 // PIECE
                        r1 = min((b + 1) * 128, (p + 1) * PIECE)
                        nc.sync.dma_start(
                            hres[r0 - b * 128 : r1 - b * 128, b * 128 : (b + 1) * 128],
                            ag3in[p][r0 - p * PIECE : r1 - p * PIECE, :],
                        )
                        r0 = r1

                for call in meta["pair_calls"]:
                    m = call["ntiles"]
                    t0 = call["tcol"]
                    inc0 = call["inc0"]
                    nincs = call["nincs"]
                    ib = mp.tile([128, m * 8], i16, tag="pib")
                    nc.sync.dma_start(ib[:, :], pbidx_t[:, t0 * 8 : (t0 + m) * 8])
                    gb = gp.tile([128, m * 128], bf16, tag="gb")
                    nc.gpsimd.dma_gather(
                        g3(gb[:, :], m),
                        tab3[call["chunk"]][:, :],
                        ib[:, :],
                        m * 128,
                        m * 128,
                        D,
                        single_packet=False,
                        queue_num=next_queue(),
                    )
                    if nincs > 0:
                        ohc = ohp.tile([128, nincs * 128], bf16, tag="ohc")
                        nc.sync.dma_start(
                            ohc[:, :], paoh_t[:, inc0 * 128 : (inc0 + nincs) * 128]
                        )
                    for bt in range(0, m, 4):
                        nb = min(4, m - bt)
                        z = zp.tile([128, 512], bf16, tag="z")
                        for i in range(nb):
                            incs = call["incs"][bt + i]
                            pa_ps = pap.tile([128, 128], f32, tag="pa")
                            for k, (ab, inc_i) in enumerate(incs):
                                nc.tensor.matmul(
                                    pa_ps[:, :],
                                    lhsT=ohc[:, (inc_i - inc0) * 128 : (inc_i - inc0 + 1) * 128],
                                    rhs=hres[:, ab * 128 : (ab + 1) * 128],
                                    start=(k == 0),
                                    stop=(k == len(incs) - 1),
                                )
                            ha = ep.tile([128, 128], bf16, tag="ha")
                            if incs:
                                nc.vector.tensor_copy(ha[:, :], pa_ps[:, :])
                            else:
                                nc.vector.memset(ha[:, :], 0.0)
                            nc.vector.tensor_mul(
                                z[:, i * 128 : (i + 1) * 128],
                                ha[:, :],
                                gb[:, (bt + i) * 128 : (bt + i + 1) * 128],
                            )
                        zt_ps = ztp.tile([128, 512], bf16, tag="zt")
                        for i in range(nb):
                            nc.tensor.matmul(
                                zt_ps[:, i * 128 : (i + 1) * 128],
                                lhsT=z[:, i * 128 : (i + 1) * 128],
                                rhs=identb[:, :],
                                is_transpose=True,
                                start=(i == 0),
                                stop=(i == nb - 1),
                            )
                        zt = ep.tile([128, 512], bf16, tag="zts")
                        nc.vector.tensor_copy(zt[:, : nb * 128], zt_ps[:, : nb * 128])
                        za_ps = zap.tile([128, 512], f32, tag="za")
                        for i in range(nb):
                            nc.tensor.matmul(
                                za_ps[:, i * 128 : (i + 1) * 128],
                                lhsT=wa[:, :],
                                rhs=zt[:, i * 128 : (i + 1) * 128],
                                start=(i == 0),
                                stop=(i == nb - 1),
                            )
                        za = ep.tile([128, 512], bf16, tag="zas")
                        nc.vector.tensor_scalar(
                            za[:, : nb * 128],
                            za_ps[:, : nb * 128],
                            ba[:, :],
                            0.0,
                            op0=aadd,
                            op1=amax,
                        )
                        sc_ps = scp.tile([1, 512], f32, tag="sc")
                        for i in range(nb):
                            nc.tensor.matmul(
                                sc_ps[:, i * 128 : (i + 1) * 128],
                                lhsT=wb[:, :],
                                rhs=za[:, i * 128 : (i + 1) * 128],
                                start=(i == 0),
                                stop=(i == nb - 1),
                            )
                        sc = ep.tile([1, 512], f32, tag="scs")
                        nc.vector.tensor_scalar_add(
                            sc[:, : nb * 128], sc_ps[:, : nb * 128], float(bb_val)
                        )
                        o0 = (t0 + bt) * 128
                        nc.sync.dma_start(
                            out_t[o0 : o0 + nb * 128].rearrange("(x n) -> x n", x=1),
                            sc[:, : nb * 128],
                        )
    nc.compile()
    return nc


# ---------------------------------------------------------------- entry point
def kernel(
    node_ids,
    senders,
    receivers,
    pairs,
    emb,
    W1,
    b1,
    W2,
    b2,
    Wa,
    ba,
    Wb,
    bb,
):
    global _LAST_EXEC_NS, _LAST_RESULTS
    from concourse import bass_utils

    node_ids = np.asarray(node_ids)
    senders = np.asarray(senders).astype(np.int64)
    receivers = np.asarray(receivers).astype(np.int64)
    pairs_np = np.asarray(pairs).astype(np.int64)
    emb = np.asarray(emb, dtype=np.float32)
    W1 = np.asarray(W1, dtype=np.float32)
    b1 = np.asarray(b1, dtype=np.float32)
    W2 = np.asarray(W2, dtype=np.float32)
    b2 = np.asarray(b2, dtype=np.float32)
    Wa = np.asarray(Wa, dtype=np.float32)
    ba = np.asarray(ba, dtype=np.float32)
    Wb = np.asarray(Wb, dtype=np.float32)
    bb = np.asarray(bb, dtype=np.float32)

    # x = emb[node_ids]
    x0 = emb[np.asarray(node_ids).astype(np.int64)]

    meta, data = _preprocess(senders, receivers, pairs_np)
    nc = _build(meta, float(bb.reshape(-1)[0]))

    iota = np.tile(np.arange(128, dtype=np.float32), (128, 1)).astype(BF16)
    in_maps = []
    for c in range(R):
        emb_sh = np.zeros((SHARD, D), BF16)
        emb_sh[:NLOC] = x0[c * NLOC : (c + 1) * NLOC].astype(BF16)
        in_maps.append(
            dict(
                emb_s=emb_sh,
                ssend=np.ascontiguousarray(data["ssend_sh"][c].reshape(NB, 128).T),
                srecv=np.ascontiguousarray(data["srecv_sh"][c].reshape(NB, 128).T),
                gidx=data["gidx"][c],
                gseg=data["gseg"][c],
                pbidx=data["pbidx"][c],
                paoh=data["paoh"][c],
                w1t=np.ascontiguousarray(W1[:D]).astype(BF16),
                w1b=np.ascontiguousarray(W1[D:]).astype(BF16),
                w2t=np.ascontiguousarray(W2[:D]).astype(BF16),
                w2b=np.ascontiguousarray(W2[D:]).astype(BF16),
                wa=Wa.astype(BF16),
                wb=Wb.astype(BF16),
                b1=b1.reshape(1, D).astype(BF16),
                b2=b2.reshape(1, D).astype(BF16),
                ba=ba.reshape(D, 1),
                iota=iota,
            )
        )

    res = bass_utils.run_bass_kernel_spmd(
        nc,
        in_maps,
        core_ids=list(range(R)),
        trace=bool(_TRACE),
        trace_cores=[0] if _TRACE == "light" else None,
    )
    _LAST_EXEC_NS = res.exec_time_ns
    _LAST_RESULTS = res

    P = pairs_np.shape[0]
    scores = np.zeros(P, np.float32)
    for c in range(R):
        v = np.asarray(res.results[c]["scores"])
        pm = data["posmap"][c]
        mvalid = pm >= 0
        scores[pm[mvalid]] = v[mvalid]
    return scores
